# revision 32
# baseline (speedup 1.0000x reference)
"""nn_Attention_16965120820033 — 16-head attention with Bayesian V/proj weights.

Sharding: 8 cores = 4 batches x 2 head-groups (8 heads each).
Per core, one fused flat pipeline: QKV projections (bf16), attention
(bf16 matmuls; softmax exp on ACT without max-subtraction — scores are
O(3); denominator pre-broadcast via 64 ones-columns appended to V),
output projection (f32r). Projections/out-proj ride the ACT-bound
attention stream as deadline-paced PE fillers to keep the tensor engine
dense (HAM warm). Host: Bayesian weight sampling (softplus), layout
transforms, f32r pre-rounding of p_w, pairwise partial sum + bias.

Self-contained: no sibling imports; shapes hardcoded.
"""

import os
import numpy as np

import concourse.bass as bass
import concourse.mybir as mybir
import concourse.tile as tile
from concourse import bass_utils

B, N, C = 4, 2048, 1024
H = 16
D = 64
G = 2                 # head-groups (tensor-parallel split)
CL = C // G           # 512 local channels
HL = H // G           # 8 local heads
HP = HL // 2          # 4 head-pairs
KT = C // 128         # 8 k-tiles over c_in
NT = N // 128         # 16 n-tiles (also m-tiles)
NCH = N // 512        # 4 n-chunks
SCALE = D ** -0.5

F32 = mybir.dt.float32
F32R = mybir.dt.float32r
BF16 = mybir.dt.bfloat16

LAST_EXEC_TIME_NS = None


# ---------------------------------------------------------------- host utils

def _rne_f32r(x):
    """Round fp32 array to float32r (RNE to 11 explicit mantissa bits)."""
    u = np.ascontiguousarray(x, np.float32).view(np.uint32).astype(np.uint64)
    r = ((u + 0x800 + ((u >> 12) & 1)) >> 12) << 12
    return r.astype(np.uint32).view(np.float32)


def _softplus(x):
    x = x.astype(np.float32)
    return np.maximum(x, 0).astype(np.float32) + np.log1p(
        np.exp(-np.abs(x), dtype=np.float32), dtype=np.float32)


def _ntff_shim():
    """Register the axon NTFF profile hook if the image's antenv lacks it."""
    import sys, types
    try:
        from antenv.axon_hooks import get_axon_ntff_profile_hook  # noqa: F401
        return
    except ImportError:
        pass
    try:
        import antenv
        from trn_agent_boot.trn_boot import _ntff_profile_via_ctypes
        m = types.ModuleType("antenv.axon_hooks")
        m._hook = _ntff_profile_via_ctypes('/opt/axon/libaxon_pjrt.so')
        m.set_axon_ntff_profile_hook = lambda h: setattr(m, "_hook", h)
        m.get_axon_ntff_profile_hook = lambda: m._hook
        sys.modules["antenv.axon_hooks"] = m
        antenv.axon_hooks = m
    except Exception:
        pass


def _split_excess_waits(nc, limit=1):
    """walrus codegen allows few sync-waits per instruction; offload extras
    onto preceding NoOps on the same engine (program order preserves
    semantics)."""
    n_added = 0
    for fn in nc.m.functions:
        for blk in fn.blocks:
            new_insts = []
            for inst in blk.instructions:
                lim = limit
                si = inst.sync_info
                w = list(si.on_wait) if si and si.on_wait else []
                if len(w) > lim:
                    excess, keep = w[:-lim], w[-lim:]
                    for i in range(0, len(excess), limit):
                        chunk = excess[i:i + limit]
                        nop = mybir.InstNoOp(
                            name=f"{inst.name}-waitsplit-{i}", ins=[], outs=[])
                        nop.engine = inst.engine
                        nop.sync_info = mybir.SyncInfo(on_wait=chunk, on_update=[])
                        new_insts.append(nop)
                        n_added += 1
                    si.on_wait = keep
                new_insts.append(inst)
            blk.instructions[:] = new_insts
    return n_added


# ---------------------------------------------------------------- device code

def build_nc():
    nc = bass.Bass()
    xb_d = nc.declare_dram_parameter("xb", [128, KT, N], BF16, isOutput=False)
    wq_d = nc.declare_dram_parameter("wq", [128, KT, CL], BF16, isOutput=False)
    wk_d = nc.declare_dram_parameter("wk", [128, KT, CL], BF16, isOutput=False)
    wv_d = nc.declare_dram_parameter("wv", [128, KT, CL], BF16, isOutput=False)
    pw_d = nc.declare_dram_parameter("pw", [128, HP, C], F32R, isOutput=False)
    y_d = nc.declare_dram_parameter("y", [N, C], F32, isOutput=True)

    with tile.TileContext(nc) as tc:
        with tc.tile_pool(name="persist", bufs=1) as pp, \
             tc.tile_pool(name="pproj", bufs=1) as pj, \
             tc.tile_pool(name="pP", bufs=1) as pP, \
             tc.tile_pool(name="pmisc", bufs=2) as pm, \
             tc.tile_pool(name="py", bufs=3) as py, \
             tc.tile_pool(name="pao", bufs=2) as pao, \
             tc.tile_pool(name="ps2s", bufs=2, space="PSUM") as ps2s, \
             tc.tile_pool(name="ps2av", bufs=2, space="PSUM") as ps2av, \
             tc.tile_pool(name="ps2m", bufs=2, space="PSUM") as ps2m:
            q_sb = pp.tile([128, HP, N], BF16)          # Q^T
            k_sb = pp.tile([128, HP, N], BF16)          # K^T
            v_sb = pp.tile([128, NT, HL, 2 * D], BF16)  # V | 64 ones cols
            nc.vector.memset(v_sb[:, :, :, D:2 * D], 1.0)
            pw_sb = pp.tile([128, HP, C], F32R)

            xb = [pj.tile([128, N], BF16, tag=f"xb{k}", name=f"xb{k}") for k in range(KT)]
            wq = [pj.tile([128, CL], BF16, tag=f"wq{k}", name=f"wq{k}") for k in range(KT)]
            wk = [pj.tile([128, CL], BF16, tag=f"wk{k}", name=f"wk{k}") for k in range(KT)]
            wv = [pj.tile([128, CL], BF16, tag=f"wv{k}", name=f"wv{k}") for k in range(KT)]
            # DMAs in consumption order; wv interleaved with chunk-0 xb so
            # the first v_proj matmul (needs wv[0] + xb[0][:, :128]) can
            # start after ~2 transfers instead of the full 2MB
            def dma_chunk(nch):
                for k in range(KT):
                    nc.sync.dma_start(xb[k][:, nch * 512:(nch + 1) * 512],
                                      xb_d[:, k, nch * 512:(nch + 1) * 512])
            # first-needed transfers split finer so they parallelize across
            # DMA queues (per-queue descriptor rate is the startup wall)
            for k in range(KT):
                nc.sync.dma_start(wv[k][:, 0:256], wv_d[:, k, 0:256])
                nc.sync.dma_start(wv[k][:, 256:512], wv_d[:, k, 256:512])
                nc.sync.dma_start(xb[k][:, 0:256], xb_d[:, k, 0:256])
                nc.sync.dma_start(xb[k][:, 256:512], xb_d[:, k, 256:512])
            for k in range(KT):
                nc.sync.dma_start(wk[k][:], wk_d[:, k, :])
            dma_chunk(1)
            for k in range(KT):
                nc.sync.dma_start(wq[k][:], wq_d[:, k, :])
            dma_chunk(2)
            dma_chunk(3)
            for hp in range(HP):
                nc.sync.dma_start(pw_sb[:, hp, :], pw_d[:, hp, :])

            def v_proj(mt, pool_tag=None):
                pool, tag = pool_tag or (ps2m, "m")
                ps = pool.tile([128, 512], F32, tag=tag, name=f"vp{mt}")
                for k in range(KT):
                    nc.tensor.matmul(
                        ps[:], xb[k][:, mt * 128:(mt + 1) * 128], wv[k][:],
                        start=(k == 0), stop=(k == KT - 1))
                nc.vector.tensor_copy(
                    v_sb[:, mt, :, 0:D],
                    ps[:].rearrange("p (h d) -> p h d", h=HL))

            def qk_proj(dst, w, t, nch, pool_tag=None):
                pool, tag = pool_tag or (ps2m, "m")
                ps = pool.tile([128, 512], F32, tag=tag, name=f"qk{t}_{nch}")
                for k in range(KT):
                    nc.tensor.matmul(
                        ps[:], w[k][:, t * 128:(t + 1) * 128],
                        xb[k][:, nch * 512:(nch + 1) * 512],
                        start=(k == 0), stop=(k == KT - 1))
                nc.vector.tensor_copy(
                    dst[:, t, nch * 512:(nch + 1) * 512], ps[:])

            def outproj(nch, nt, cch, pool_tag=None):
                pool, tag = pool_tag or (ps2m, "m")
                yp = pool.tile([128, 512], F32, tag=tag, name=f"y{nch}_{nt}_{cch}")
                ao = ao_tiles[nch]
                for hp in range(HP):
                    nc.tensor.matmul(
                        yp[:], ao[:, hp, nt * 128:(nt + 1) * 128],
                        pw_sb[:, hp, cch * 512:(cch + 1) * 512],
                        start=(hp == 0), stop=(hp == HP - 1))
                y_sb = py.tile([128, 512], F32, tag="y", name=f"ysb{nch}_{nt}_{cch}")
                nc.vector.tensor_copy(y_sb[:], yp[:])
                nc.sync.dma_start(
                    y_d[nch * 512 + nt * 128:nch * 512 + (nt + 1) * 128,
                        cch * 512:(cch + 1) * 512], y_sb[:])

            # warm-up work emitted densely in DMA-arrival (chunk-major)
            # order: V projection interleaved with t0 K per chunk, then t0 Q
            # for the first chunk; the rest rides as fillers
            for nch in range(NCH):
                for mt in range(4 * nch, 4 * nch + 4):
                    v_proj(mt, (ps2av, "av"))
                qk_proj(k_sb, wk, 0, nch, (ps2av, "av"))
            qk_proj(q_sb, wq, 0, 0, (ps2av, "av"))

            # per-nc filler inventories (drained evenly across that nc's slots)
            nc_fillers = {i: [] for i in range(NCH)}
            for t in range(1, HP):
                for nch in range(NCH):
                    nc_fillers[0].append((qk_proj, (k_sb, wk, t, nch)))
                nc_fillers[0].append((qk_proj, (q_sb, wq, t, 0)))
            for target_nc in range(1, NCH):
                src_nc = target_nc - 1
                for t in range(1, HP):
                    nc_fillers[src_nc].append((qk_proj, (q_sb, wq, t, target_nc)))
                nc_fillers[src_nc].append((qk_proj, (q_sb, wq, 0, target_nc)))

            ao_tiles = {}
            av_tiles = {}
            # P ring: [:, 0] = head 2hp, [:, 1] = head 2hp+1; 16 m-tile
            # slices per side reused each hp (slice-level deps)
            p_ring = pP.tile([128, 2, 16, 512], BF16, tag="pr", name="p_ring")

            units = [(nch, hp, mt) for nch in range(NCH)
                     for hp in range(HP) for mt in range(NT)]
            LAG = 2
            UPN = HP * NT                 # units per n-chunk

            def emit_qk_exp(nch, hp, mt):
                nsl = slice(nch * 512, (nch + 1) * 512)
                msl = slice(mt * 128, (mt + 1) * 128)
                s2 = ps2s.tile([128, 2, 512], F32, tag="s", name=f"s{nch}_{hp}_{mt}")
                nc.tensor.matmul(
                    s2[:, 0], k_sb[0:64, hp, msl], q_sb[0:64, hp, nsl],
                    start=True, stop=True, tile_position=(0, 0))
                nc.tensor.matmul(
                    s2[:, 1], k_sb[64:128, hp, msl], q_sb[64:128, hp, nsl],
                    start=True, stop=True, tile_position=(64, 0))
                # one exp for both heads: ap=1024, out strided over both sides
                nc.scalar.activation(
                    p_ring[:, :, mt, :], s2[:],
                    mybir.ActivationFunctionType.Exp, scale=SCALE)

            def emit_av(nch, hp, mt):
                if (nch, hp) not in av_tiles:
                    av_tiles[(nch, hp)] = (
                        ps2av.tile([128, 512], F32, tag="av", name=f"avA{nch}_{hp}"),
                        ps2av.tile([128, 512], F32, tag="av", name=f"avB{nch}_{hp}"))
                avA, avB = av_tiles[(nch, hp)]
                nc.tensor.matmul(
                    avA[:], v_sb[:, mt, 2 * hp, :], p_ring[:, 0, mt, :],
                    start=(mt == 0), stop=(mt == NT - 1))
                nc.tensor.matmul(
                    avB[:], v_sb[:, mt, 2 * hp + 1, :], p_ring[:, 1, mt, :],
                    start=(mt == 0), stop=(mt == NT - 1))

            def emit_normalize(nch, hp, final=False):
                ao_sb = ao_tiles[nch]
                avA, avB = av_tiles.pop((nch, hp))
                if final:
                    # no one reuses the av PSUM slots after the final pairs:
                    # skip the stash copies, read PSUM directly
                    srcA, srcB = avA, avB
                else:
                    # stash both sides to SBUF immediately: the two copies
                    # free the av PSUM slots the next pair rotates onto; the
                    # slow DVE reciprocals run later in small pieces
                    srcA = pm.tile([128, 512], F32, tag="avsA")
                    srcB = pm.tile([128, 512], F32, tag="avsB")
                    nc.vector.tensor_copy(srcA[:], avA[:])
                    nc.vector.tensor_copy(srcB[:], avB[:])
                rbA = pm.tile([64, 512], F32, tag="rbA")
                rbB = pm.tile([64, 512], F32, tag="rbB")
                # per n-128-slice pieces, mult right after its recips: a
                # consumer of ao[:, hp, q-slice] only waits ~4 pieces
                for q in range(4):
                    qs = slice(q * 128, (q + 1) * 128)
                    for avs, rb, base in ((srcA, rbA, 0), (srcB, rbB, 64)):
                        dve_fillers.append((nch,
                            lambda avs=avs, rb=rb, qs=qs:
                            nc.vector.reciprocal(rb[:, qs], avs[D:2 * D, qs])))
                        dve_fillers.append((nch,
                            lambda avs=avs, rb=rb, base=base, hp=hp, qs=qs:
                            nc.vector.tensor_tensor(
                                ao_sb[base:base + 64, hp, qs], avs[0:D, qs],
                                rb[:, qs], mybir.AluOpType.mult)))
                if hp == HP - 1:
                    if nch < NCH - 1:
                        for nt in range(4):
                            for cch in range(2):
                                nc_fillers[nch + 1].append((outproj, (nch, nt, cch)))
                    else:
                        for nt in range(4):
                            for cch in range(2):
                                nc_fillers[NCH - 1].append((outproj, (nch, nt, cch)))

            drained_nc = {i: 0 for i in range(NCH)}
            dve_fillers = []
            pieces_clear_ui = {}
            for ui, (nch, hp, mt) in enumerate(units):
                if hp == 0 and mt == 0:
                    ao_tiles[nch] = pao.tile([128, HP, 512], F32R, tag="ao",
                                             name=f"ao{nch}")
                # ready work first (AV of lagged unit + fillers), scores last
                if ui >= LAG:
                    lnch, lhp, lmt = units[ui - LAG]
                    emit_av(lnch, lhp, lmt)
                    if lmt == NT - 1:
                        emit_normalize(lnch, lhp)
                # paced filler drain (inventory of the CURRENT nc)
                fl = nc_fillers[nch]
                slot_in_nc = ui % UPN + 1
                inv = len(fl) + drained_nc[nch]
                want = (inv * slot_in_nc + UPN - 1) // UPN
                while fl and drained_nc[nch] < want:
                    f, args = fl.pop(0)
                    if f is outproj and (
                            any(c == args[0] for c, _ in dve_fillers)
                            or ui < pieces_clear_ui.get(args[0], 0) + 2):
                        # outproj reads ao written by deferred normalize
                        # pieces of ITS chunk; hold until those are emitted
                        # AND the DVE has had ~2 units to execute them
                        fl.insert(0, (f, args))
                        break
                    f(*args)
                    drained_nc[nch] += 1
                for _ in range(2):
                    if dve_fillers:
                        c, fn = dve_fillers.pop(0)
                        fn()
                        if not any(cc == c for cc, _ in dve_fillers):
                            pieces_clear_ui[c] = ui
                emit_qk_exp(nch, hp, mt)
            for ui in range(len(units) - LAG, len(units)):
                nch, hp, mt = units[ui]
                emit_av(nch, hp, mt)
                if mt == NT - 1:
                    emit_normalize(nch, hp, final=True)
            while dve_fillers:
                dve_fillers.pop(0)[1]()
            # tail outprojs anchor on the "s"/"av" PSUM tags whose previous
            # users are the final exps/AVs: the WAR on the tile allocation
            # stops walrus hoisting these matmuls early into the PE stream,
            # where their not-yet-written ao stationary would stall the PE
            tail_pools = [(ps2s, "s"), (ps2av, "av")]
            ti = 0
            for i in range(NCH):
                for f, args in nc_fillers[i]:
                    if f is outproj:
                        f(*args, pool_tag=tail_pools[ti % 2])
                        ti += 1
                    else:
                        f(*args)
    return nc


# ---------------------------------------------------------------- entry point

def kernel(x, q_w, k_w, v_mu, v_rho, v_eps, proj_mu, proj_rho, proj_eps,
           pb_mu, pb_rho, pb_eps):
    global LAST_EXEC_TIME_NS
    _ntff_shim()
    import ml_dtypes

    x = np.asarray(x, np.float32)
    v_w = (np.asarray(v_mu, np.float32)
           + _softplus(np.asarray(v_rho)) * np.asarray(v_eps, np.float32))
    p_w = (np.asarray(proj_mu, np.float32)
           + _softplus(np.asarray(proj_rho)) * np.asarray(proj_eps, np.float32))
    p_b = (np.asarray(pb_mu, np.float32)
           + _softplus(np.asarray(pb_rho)) * np.asarray(pb_eps, np.float32))

    def wslice(w, g):  # [128, KT, CL] bf16 layout of w[g*CL:(g+1)*CL, :].T
        wt = np.ascontiguousarray(np.asarray(w, np.float32)[g * CL:(g + 1) * CL, :].T)
        return wt.reshape(KT, 128, CL).transpose(1, 0, 2).astype(ml_dtypes.bfloat16)

    def pwslice(g):    # [128, HP, C] f32r layout of p_w[:, g*CL:(g+1)*CL].T
        pt = np.ascontiguousarray(p_w[:, g * CL:(g + 1) * CL].T)
        return _rne_f32r(pt.reshape(HP, 128, C).transpose(1, 0, 2))

    xts = []
    for b in range(B):
        xt = np.ascontiguousarray(x[b].T)          # [C, N]
        xts.append(xt.reshape(KT, 128, N).transpose(1, 0, 2).astype(ml_dtypes.bfloat16))
    wq = [wslice(q_w, g) for g in range(G)]
    wk = [wslice(k_w, g) for g in range(G)]
    wv = [wslice(v_w, g) for g in range(G)]
    pw = [pwslice(g) for g in range(G)]

    in_maps = []
    for core in range(8):
        b, g = core // 2, core % 2
        in_maps.append({"xb": xts[b], "wq": wq[g], "wk": wk[g],
                        "wv": wv[g], "pw": pw[g]})

    nc = build_nc()
    _split_excess_waits(nc)
    res = bass_utils.run_bass_kernel_spmd(
        nc, in_maps, core_ids=list(range(8)),
        trace=bool(os.environ.get("BASS_TRACE")))
    LAST_EXEC_TIME_NS = res.exec_time_ns

    out = np.empty((B, N, C), np.float32)
    for b in range(B):
        out[b] = res.results[2 * b]["y"] + res.results[2 * b + 1]["y"] + p_b
    return out



# revision 33
# speedup vs baseline: 1.0496x; 1.0496x over previous
"""nn_Attention_16965120820033 — 16-head attention with Bayesian V/proj weights.

Sharding: 8 cores = 4 batches x 2 head-groups (8 heads each).
Per core, one fused flat pipeline: QKV projections (bf16), attention
(bf16 matmuls; softmax exp on ACT without max-subtraction — scores are
O(3); denominator pre-broadcast via 64 ones-columns appended to V),
output projection (f32r). Projections/out-proj ride the ACT-bound
attention stream as deadline-paced PE fillers to keep the tensor engine
dense (HAM warm). Host: Bayesian weight sampling (softplus), layout
transforms, f32r pre-rounding of p_w, pairwise partial sum + bias.

Self-contained: no sibling imports; shapes hardcoded.
"""

import os
import numpy as np

import concourse.bass as bass
import concourse.mybir as mybir
import concourse.tile as tile
from concourse import bass_utils

B, N, C = 4, 2048, 1024
H = 16
D = 64
G = 2                 # head-groups (tensor-parallel split)
CL = C // G           # 512 local channels
HL = H // G           # 8 local heads
HP = HL // 2          # 4 head-pairs
KT = C // 128         # 8 k-tiles over c_in
NT = N // 128         # 16 n-tiles (also m-tiles)
NCH = N // 512        # 4 n-chunks
SCALE = D ** -0.5

F32 = mybir.dt.float32
F32R = mybir.dt.float32r
BF16 = mybir.dt.bfloat16

LAST_EXEC_TIME_NS = None


# ---------------------------------------------------------------- host utils

def _rne_f32r(x):
    """Round fp32 array to float32r (RNE to 11 explicit mantissa bits)."""
    u = np.ascontiguousarray(x, np.float32).view(np.uint32).astype(np.uint64)
    r = ((u + 0x800 + ((u >> 12) & 1)) >> 12) << 12
    return r.astype(np.uint32).view(np.float32)


def _softplus(x):
    x = x.astype(np.float32)
    return np.maximum(x, 0).astype(np.float32) + np.log1p(
        np.exp(-np.abs(x), dtype=np.float32), dtype=np.float32)


def _ntff_shim():
    """Register the axon NTFF profile hook if the image's antenv lacks it."""
    import sys, types
    try:
        from antenv.axon_hooks import get_axon_ntff_profile_hook  # noqa: F401
        return
    except ImportError:
        pass
    try:
        import antenv
        from trn_agent_boot.trn_boot import _ntff_profile_via_ctypes
        m = types.ModuleType("antenv.axon_hooks")
        m._hook = _ntff_profile_via_ctypes('/opt/axon/libaxon_pjrt.so')
        m.set_axon_ntff_profile_hook = lambda h: setattr(m, "_hook", h)
        m.get_axon_ntff_profile_hook = lambda: m._hook
        sys.modules["antenv.axon_hooks"] = m
        antenv.axon_hooks = m
    except Exception:
        pass


def _split_excess_waits(nc, limit=1):
    """walrus codegen allows few sync-waits per instruction; offload extras
    onto preceding NoOps on the same engine (program order preserves
    semantics)."""
    n_added = 0
    for fn in nc.m.functions:
        for blk in fn.blocks:
            new_insts = []
            for inst in blk.instructions:
                lim = limit
                si = inst.sync_info
                w = list(si.on_wait) if si and si.on_wait else []
                if len(w) > lim:
                    excess, keep = w[:-lim], w[-lim:]
                    for i in range(0, len(excess), limit):
                        chunk = excess[i:i + limit]
                        nop = mybir.InstNoOp(
                            name=f"{inst.name}-waitsplit-{i}", ins=[], outs=[])
                        nop.engine = inst.engine
                        nop.sync_info = mybir.SyncInfo(on_wait=chunk, on_update=[])
                        new_insts.append(nop)
                        n_added += 1
                    si.on_wait = keep
                new_insts.append(inst)
            blk.instructions[:] = new_insts
    return n_added


# ---------------------------------------------------------------- device code

def build_nc():
    nc = bass.Bass()
    xb_d = nc.declare_dram_parameter("xb", [128, KT, N], BF16, isOutput=False)
    wq_d = nc.declare_dram_parameter("wq", [128, KT, CL], BF16, isOutput=False)
    wk_d = nc.declare_dram_parameter("wk", [128, KT, CL], BF16, isOutput=False)
    wv_d = nc.declare_dram_parameter("wv", [128, KT, CL], BF16, isOutput=False)
    pw_d = nc.declare_dram_parameter("pw", [128, HP, C], F32R, isOutput=False)
    y_d = nc.declare_dram_parameter("y", [N, C], F32, isOutput=True)

    with tile.TileContext(nc) as tc:
        with tc.tile_pool(name="persist", bufs=1) as pp, \
             tc.tile_pool(name="pproj", bufs=1) as pj, \
             tc.tile_pool(name="pP", bufs=1) as pP, \
             tc.tile_pool(name="pmisc", bufs=2) as pm, \
             tc.tile_pool(name="py", bufs=3) as py, \
             tc.tile_pool(name="pao", bufs=2) as pao, \
             tc.tile_pool(name="ps2s", bufs=2, space="PSUM") as ps2s, \
             tc.tile_pool(name="ps2av", bufs=2, space="PSUM") as ps2av, \
             tc.tile_pool(name="ps2m", bufs=2, space="PSUM") as ps2m:
            q_sb = pp.tile([128, HP, N], BF16)          # Q^T
            k_sb = pp.tile([128, HP, N], BF16)          # K^T
            v_sb = pp.tile([128, NT, HL, 2 * D], BF16)  # V | 64 ones cols
            nc.vector.memset(v_sb[:, :, :, D:2 * D], 1.0)
            pw_sb = pp.tile([128, HP, C], F32R)

            xb = [pj.tile([128, N], BF16, tag=f"xb{k}", name=f"xb{k}") for k in range(KT)]
            wq = [pj.tile([128, CL], BF16, tag=f"wq{k}", name=f"wq{k}") for k in range(KT)]
            wk = [pj.tile([128, CL], BF16, tag=f"wk{k}", name=f"wk{k}") for k in range(KT)]
            wv = [pj.tile([128, CL], BF16, tag=f"wv{k}", name=f"wv{k}") for k in range(KT)]
            # DMAs in consumption order; wv interleaved with chunk-0 xb so
            # the first v_proj matmul (needs wv[0] + xb[0][:, :128]) can
            # start after ~2 transfers instead of the full 2MB
            def dma_chunk(nch):
                for k in range(KT):
                    nc.sync.dma_start(xb[k][:, nch * 512:(nch + 1) * 512],
                                      xb_d[:, k, nch * 512:(nch + 1) * 512])
            # first-needed transfers split finer so they parallelize across
            # DMA queues (per-queue descriptor rate is the startup wall)
            for k in range(KT):
                nc.sync.dma_start(wv[k][:, 0:256], wv_d[:, k, 0:256])
                nc.sync.dma_start(wv[k][:, 256:512], wv_d[:, k, 256:512])
                nc.sync.dma_start(xb[k][:, 0:256], xb_d[:, k, 0:256])
                nc.sync.dma_start(xb[k][:, 256:512], xb_d[:, k, 256:512])
            for k in range(KT):
                nc.sync.dma_start(wk[k][:], wk_d[:, k, :])
            dma_chunk(1)
            for k in range(KT):
                nc.sync.dma_start(wq[k][:], wq_d[:, k, :])
            dma_chunk(2)
            dma_chunk(3)
            for hp in range(HP):
                nc.sync.dma_start(pw_sb[:, hp, :], pw_d[:, hp, :])

            def v_proj(mt, pool_tag=None):
                pool, tag = pool_tag or (ps2m, "m")
                ps = pool.tile([128, 512], F32, tag=tag, name=f"vp{mt}")
                for k in range(KT):
                    nc.tensor.matmul(
                        ps[:], xb[k][:, mt * 128:(mt + 1) * 128], wv[k][:],
                        start=(k == 0), stop=(k == KT - 1))
                nc.vector.tensor_copy(
                    v_sb[:, mt, :, 0:D],
                    ps[:].rearrange("p (h d) -> p h d", h=HL))

            def qk_proj(dst, w, t, nch, pool_tag=None):
                pool, tag = pool_tag or (ps2m, "m")
                ps = pool.tile([128, 512], F32, tag=tag, name=f"qk{t}_{nch}")
                for k in range(KT):
                    nc.tensor.matmul(
                        ps[:], w[k][:, t * 128:(t + 1) * 128],
                        xb[k][:, nch * 512:(nch + 1) * 512],
                        start=(k == 0), stop=(k == KT - 1))
                nc.vector.tensor_copy(
                    dst[:, t, nch * 512:(nch + 1) * 512], ps[:])

            def outproj(nch, nt, cch, pool_tag=None):
                pool, tag = pool_tag or (ps2m, "m")
                yp = pool.tile([128, 512], F32, tag=tag, name=f"y{nch}_{nt}_{cch}")
                ao = ao_tiles[nch]
                for hp in range(HP):
                    nc.tensor.matmul(
                        yp[:], ao[:, hp, nt * 128:(nt + 1) * 128],
                        pw_sb[:, hp, cch * 512:(cch + 1) * 512],
                        start=(hp == 0), stop=(hp == HP - 1))
                y_sb = py.tile([128, 512], F32, tag="y", name=f"ysb{nch}_{nt}_{cch}")
                nc.vector.tensor_copy(y_sb[:], yp[:])
                nc.sync.dma_start(
                    y_d[nch * 512 + nt * 128:nch * 512 + (nt + 1) * 128,
                        cch * 512:(cch + 1) * 512], y_sb[:])

            # warm-up work emitted densely in DMA-arrival (chunk-major)
            # order: V projection interleaved with t0 K per chunk, then t0 Q
            # for the first chunk; the rest rides as fillers
            for nch in range(NCH):
                for mt in range(4 * nch, 4 * nch + 4):
                    v_proj(mt, (ps2av, "av"))
                qk_proj(k_sb, wk, 0, nch, (ps2av, "av"))
            qk_proj(q_sb, wq, 0, 0, (ps2av, "av"))

            # per-nc filler inventories (drained evenly across that nc's slots)
            nc_fillers = {i: [] for i in range(NCH)}
            for t in range(1, HP):
                for nch in range(NCH):
                    nc_fillers[0].append((qk_proj, (k_sb, wk, t, nch)))
                nc_fillers[0].append((qk_proj, (q_sb, wq, t, 0)))
            for target_nc in range(1, NCH):
                src_nc = target_nc - 1
                for t in range(1, HP):
                    nc_fillers[src_nc].append((qk_proj, (q_sb, wq, t, target_nc)))
                nc_fillers[src_nc].append((qk_proj, (q_sb, wq, 0, target_nc)))

            ao_tiles = {}
            av_tiles = {}
            # P ring: [:, 0] = head 2hp, [:, 1] = head 2hp+1; 16 m-tile
            # slices per side reused each hp (slice-level deps)
            p_ring = pP.tile([128, 2, 16, 512], BF16, tag="pr", name="p_ring")

            units = [(nch, hp, mt) for nch in range(NCH)
                     for hp in range(HP) for mt in range(NT)]
            LAG = 3
            UPN = HP * NT                 # units per n-chunk

            def emit_qk_exp(nch, hp, mt):
                nsl = slice(nch * 512, (nch + 1) * 512)
                msl = slice(mt * 128, (mt + 1) * 128)
                s2 = ps2s.tile([128, 2, 512], F32, tag="s", name=f"s{nch}_{hp}_{mt}")
                nc.tensor.matmul(
                    s2[:, 0], k_sb[0:64, hp, msl], q_sb[0:64, hp, nsl],
                    start=True, stop=True, tile_position=(0, 0))
                nc.tensor.matmul(
                    s2[:, 1], k_sb[64:128, hp, msl], q_sb[64:128, hp, nsl],
                    start=True, stop=True, tile_position=(64, 0))
                # one exp for both heads: ap=1024, out strided over both sides
                nc.scalar.activation(
                    p_ring[:, :, mt, :], s2[:],
                    mybir.ActivationFunctionType.Exp, scale=SCALE)

            def emit_av(nch, hp, mt):
                if (nch, hp) not in av_tiles:
                    av_tiles[(nch, hp)] = (
                        ps2av.tile([128, 512], F32, tag="av", name=f"avA{nch}_{hp}"),
                        ps2av.tile([128, 512], F32, tag="av", name=f"avB{nch}_{hp}"))
                avA, avB = av_tiles[(nch, hp)]
                nc.tensor.matmul(
                    avA[:], v_sb[:, mt, 2 * hp, :], p_ring[:, 0, mt, :],
                    start=(mt == 0), stop=(mt == NT - 1))
                nc.tensor.matmul(
                    avB[:], v_sb[:, mt, 2 * hp + 1, :], p_ring[:, 1, mt, :],
                    start=(mt == 0), stop=(mt == NT - 1))

            def emit_normalize(nch, hp, final=False):
                ao_sb = ao_tiles[nch]
                avA, avB = av_tiles.pop((nch, hp))
                if final:
                    # no one reuses the av PSUM slots after the final pairs:
                    # skip the stash copies, read PSUM directly
                    srcA, srcB = avA, avB
                else:
                    # stash both sides to SBUF immediately: the two copies
                    # free the av PSUM slots the next pair rotates onto; the
                    # slow DVE reciprocals run later in small pieces
                    srcA = pm.tile([128, 512], F32, tag="avsA")
                    srcB = pm.tile([128, 512], F32, tag="avsB")
                    nc.vector.tensor_copy(srcA[:], avA[:])
                    nc.vector.tensor_copy(srcB[:], avB[:])
                rbA = pm.tile([64, 512], F32, tag="rbA")
                rbB = pm.tile([64, 512], F32, tag="rbB")
                # per n-128-slice pieces, mult right after its recips: a
                # consumer of ao[:, hp, q-slice] only waits ~4 pieces
                for q in range(4):
                    qs = slice(q * 128, (q + 1) * 128)
                    for avs, rb, base in ((srcA, rbA, 0), (srcB, rbB, 64)):
                        dve_fillers.append((nch,
                            lambda avs=avs, rb=rb, qs=qs:
                            nc.vector.reciprocal(rb[:, qs], avs[D:2 * D, qs])))
                        dve_fillers.append((nch,
                            lambda avs=avs, rb=rb, base=base, hp=hp, qs=qs:
                            nc.vector.tensor_tensor(
                                ao_sb[base:base + 64, hp, qs], avs[0:D, qs],
                                rb[:, qs], mybir.AluOpType.mult)))
                if hp == HP - 1:
                    if nch < NCH - 1:
                        for nt in range(4):
                            for cch in range(2):
                                nc_fillers[nch + 1].append((outproj, (nch, nt, cch)))
                    else:
                        for nt in range(4):
                            for cch in range(2):
                                nc_fillers[NCH - 1].append((outproj, (nch, nt, cch)))

            drained_nc = {i: 0 for i in range(NCH)}
            dve_fillers = []
            pieces_clear_ui = {}
            for ui, (nch, hp, mt) in enumerate(units):
                if hp == 0 and mt == 0:
                    ao_tiles[nch] = pao.tile([128, HP, 512], F32R, tag="ao",
                                             name=f"ao{nch}")
                # ready work first (AV of lagged unit + fillers), scores last
                if ui >= LAG:
                    lnch, lhp, lmt = units[ui - LAG]
                    emit_av(lnch, lhp, lmt)
                    if lmt == NT - 1:
                        emit_normalize(lnch, lhp)
                # paced filler drain (inventory of the CURRENT nc)
                fl = nc_fillers[nch]
                slot_in_nc = ui % UPN + 1
                inv = len(fl) + drained_nc[nch]
                want = (inv * slot_in_nc + UPN - 1) // UPN
                while fl and drained_nc[nch] < want:
                    f, args = fl.pop(0)
                    if f is outproj and (
                            any(c == args[0] for c, _ in dve_fillers)
                            or ui < pieces_clear_ui.get(args[0], 0) + 2):
                        # outproj reads ao written by deferred normalize
                        # pieces of ITS chunk; hold until those are emitted
                        # AND the DVE has had ~2 units to execute them
                        fl.insert(0, (f, args))
                        break
                    f(*args)
                    drained_nc[nch] += 1
                for _ in range(2):
                    if dve_fillers:
                        c, fn = dve_fillers.pop(0)
                        fn()
                        if not any(cc == c for cc, _ in dve_fillers):
                            pieces_clear_ui[c] = ui
                emit_qk_exp(nch, hp, mt)
            for ui in range(len(units) - LAG, len(units)):
                nch, hp, mt = units[ui]
                emit_av(nch, hp, mt)
                if mt == NT - 1:
                    emit_normalize(nch, hp, final=True)
            while dve_fillers:
                dve_fillers.pop(0)[1]()
            # tail outprojs anchor on the "s"/"av" PSUM tags whose previous
            # users are the final exps/AVs: the WAR on the tile allocation
            # stops walrus hoisting these matmuls early into the PE stream,
            # where their not-yet-written ao stationary would stall the PE
            tail_pools = [(ps2s, "s"), (ps2av, "av")]
            ti = 0
            for i in range(NCH):
                for f, args in nc_fillers[i]:
                    if f is outproj:
                        f(*args, pool_tag=tail_pools[ti % 2])
                        ti += 1
                    else:
                        f(*args)
    return nc


# ---------------------------------------------------------------- entry point

def kernel(x, q_w, k_w, v_mu, v_rho, v_eps, proj_mu, proj_rho, proj_eps,
           pb_mu, pb_rho, pb_eps):
    global LAST_EXEC_TIME_NS
    _ntff_shim()
    import ml_dtypes

    x = np.asarray(x, np.float32)
    v_w = (np.asarray(v_mu, np.float32)
           + _softplus(np.asarray(v_rho)) * np.asarray(v_eps, np.float32))
    p_w = (np.asarray(proj_mu, np.float32)
           + _softplus(np.asarray(proj_rho)) * np.asarray(proj_eps, np.float32))
    p_b = (np.asarray(pb_mu, np.float32)
           + _softplus(np.asarray(pb_rho)) * np.asarray(pb_eps, np.float32))

    def wslice(w, g):  # [128, KT, CL] bf16 layout of w[g*CL:(g+1)*CL, :].T
        wt = np.ascontiguousarray(np.asarray(w, np.float32)[g * CL:(g + 1) * CL, :].T)
        return wt.reshape(KT, 128, CL).transpose(1, 0, 2).astype(ml_dtypes.bfloat16)

    def pwslice(g):    # [128, HP, C] f32r layout of p_w[:, g*CL:(g+1)*CL].T
        pt = np.ascontiguousarray(p_w[:, g * CL:(g + 1) * CL].T)
        return _rne_f32r(pt.reshape(HP, 128, C).transpose(1, 0, 2))

    xts = []
    for b in range(B):
        xt = np.ascontiguousarray(x[b].T)          # [C, N]
        xts.append(xt.reshape(KT, 128, N).transpose(1, 0, 2).astype(ml_dtypes.bfloat16))
    wq = [wslice(q_w, g) for g in range(G)]
    wk = [wslice(k_w, g) for g in range(G)]
    wv = [wslice(v_w, g) for g in range(G)]
    pw = [pwslice(g) for g in range(G)]

    in_maps = []
    for core in range(8):
        b, g = core // 2, core % 2
        in_maps.append({"xb": xts[b], "wq": wq[g], "wk": wk[g],
                        "wv": wv[g], "pw": pw[g]})

    nc = build_nc()
    _split_excess_waits(nc)
    res = bass_utils.run_bass_kernel_spmd(
        nc, in_maps, core_ids=list(range(8)),
        trace=bool(os.environ.get("BASS_TRACE")))
    LAST_EXEC_TIME_NS = res.exec_time_ns

    out = np.empty((B, N, C), np.float32)
    for b in range(B):
        out[b] = res.results[2 * b]["y"] + res.results[2 * b + 1]["y"] + p_b
    return out



# revision 34
# speedup vs baseline: 1.0850x; 1.0337x over previous
"""nn_Attention_16965120820033 — 16-head attention with Bayesian V/proj weights.

Sharding: 8 cores = 4 batches x 2 head-groups (8 heads each).
Per core, one fused flat pipeline: QKV projections (bf16), attention
(bf16 matmuls; softmax exp on ACT without max-subtraction — scores are
O(3); denominator pre-broadcast via 64 ones-columns appended to V),
output projection (f32r). Projections/out-proj ride the ACT-bound
attention stream as deadline-paced PE fillers to keep the tensor engine
dense (HAM warm). Host: Bayesian weight sampling (softplus), layout
transforms, f32r pre-rounding of p_w, pairwise partial sum + bias.

Self-contained: no sibling imports; shapes hardcoded.
"""

import os
import numpy as np

import concourse.bass as bass
import concourse.mybir as mybir
import concourse.tile as tile
from concourse import bass_utils

B, N, C = 4, 2048, 1024
H = 16
D = 64
G = 2                 # head-groups (tensor-parallel split)
CL = C // G           # 512 local channels
HL = H // G           # 8 local heads
HP = HL // 2          # 4 head-pairs
KT = C // 128         # 8 k-tiles over c_in
NT = N // 128         # 16 n-tiles (also m-tiles)
NCH = N // 512        # 4 n-chunks
SCALE = D ** -0.5

F32 = mybir.dt.float32
F32R = mybir.dt.float32r
BF16 = mybir.dt.bfloat16

LAST_EXEC_TIME_NS = None


# ---------------------------------------------------------------- host utils

def _rne_f32r(x):
    """Round fp32 array to float32r (RNE to 11 explicit mantissa bits)."""
    u = np.ascontiguousarray(x, np.float32).view(np.uint32).astype(np.uint64)
    r = ((u + 0x800 + ((u >> 12) & 1)) >> 12) << 12
    return r.astype(np.uint32).view(np.float32)


def _softplus(x):
    x = x.astype(np.float32)
    return np.maximum(x, 0).astype(np.float32) + np.log1p(
        np.exp(-np.abs(x), dtype=np.float32), dtype=np.float32)


def _ntff_shim():
    """Register the axon NTFF profile hook if the image's antenv lacks it."""
    import sys, types
    try:
        from antenv.axon_hooks import get_axon_ntff_profile_hook  # noqa: F401
        return
    except ImportError:
        pass
    try:
        import antenv
        from trn_agent_boot.trn_boot import _ntff_profile_via_ctypes
        m = types.ModuleType("antenv.axon_hooks")
        m._hook = _ntff_profile_via_ctypes('/opt/axon/libaxon_pjrt.so')
        m.set_axon_ntff_profile_hook = lambda h: setattr(m, "_hook", h)
        m.get_axon_ntff_profile_hook = lambda: m._hook
        sys.modules["antenv.axon_hooks"] = m
        antenv.axon_hooks = m
    except Exception:
        pass


def _split_excess_waits(nc, limit=1):
    """walrus codegen allows few sync-waits per instruction; offload extras
    onto preceding NoOps on the same engine (program order preserves
    semantics)."""
    n_added = 0
    for fn in nc.m.functions:
        for blk in fn.blocks:
            new_insts = []
            for inst in blk.instructions:
                lim = limit
                si = inst.sync_info
                w = list(si.on_wait) if si and si.on_wait else []
                if len(w) > lim:
                    excess, keep = w[:-lim], w[-lim:]
                    for i in range(0, len(excess), limit):
                        chunk = excess[i:i + limit]
                        nop = mybir.InstNoOp(
                            name=f"{inst.name}-waitsplit-{i}", ins=[], outs=[])
                        nop.engine = inst.engine
                        nop.sync_info = mybir.SyncInfo(on_wait=chunk, on_update=[])
                        new_insts.append(nop)
                        n_added += 1
                    si.on_wait = keep
                new_insts.append(inst)
            blk.instructions[:] = new_insts
    return n_added


# ---------------------------------------------------------------- device code

def build_nc():
    nc = bass.Bass()
    xb_d = nc.declare_dram_parameter("xb", [128, KT, N], BF16, isOutput=False)
    wq_d = nc.declare_dram_parameter("wq", [128, KT, CL], BF16, isOutput=False)
    wk_d = nc.declare_dram_parameter("wk", [128, KT, CL], BF16, isOutput=False)
    wv_d = nc.declare_dram_parameter("wv", [128, KT, CL], BF16, isOutput=False)
    pw_d = nc.declare_dram_parameter("pw", [128, HP, C], F32R, isOutput=False)
    y_d = nc.declare_dram_parameter("y", [N, C], F32, isOutput=True)

    with tile.TileContext(nc) as tc:
        with tc.tile_pool(name="persist", bufs=1) as pp, \
             tc.tile_pool(name="pproj", bufs=1) as pj, \
             tc.tile_pool(name="pP", bufs=1) as pP, \
             tc.tile_pool(name="pmisc", bufs=2) as pm, \
             tc.tile_pool(name="py", bufs=3) as py, \
             tc.tile_pool(name="pao", bufs=2) as pao, \
             tc.tile_pool(name="ps2s", bufs=2, space="PSUM") as ps2s, \
             tc.tile_pool(name="ps2av", bufs=2, space="PSUM") as ps2av, \
             tc.tile_pool(name="ps2m", bufs=2, space="PSUM") as ps2m:
            q_sb = pp.tile([128, HP, N], BF16)          # Q^T
            k_sb = pp.tile([128, HP, N], BF16)          # K^T
            v_sb = pp.tile([128, NT, HL, 2 * D], BF16)  # V | 64 ones cols
            nc.vector.memset(v_sb[:, :, :, D:2 * D], 1.0)
            pw_sb = pp.tile([128, HP, C], F32R)

            xb = [pj.tile([128, N], BF16, tag=f"xb{k}", name=f"xb{k}") for k in range(KT)]
            wq = [pj.tile([128, CL], BF16, tag=f"wq{k}", name=f"wq{k}") for k in range(KT)]
            wk = [pj.tile([128, CL], BF16, tag=f"wk{k}", name=f"wk{k}") for k in range(KT)]
            wv = [pj.tile([128, CL], BF16, tag=f"wv{k}", name=f"wv{k}") for k in range(KT)]
            # DMAs in consumption order; wv interleaved with chunk-0 xb so
            # the first v_proj matmul (needs wv[0] + xb[0][:, :128]) can
            # start after ~2 transfers instead of the full 2MB
            def dma_chunk(nch):
                for k in range(KT):
                    nc.sync.dma_start(xb[k][:, nch * 512:(nch + 1) * 512],
                                      xb_d[:, k, nch * 512:(nch + 1) * 512])
            # first-needed transfers split finer so they parallelize across
            # DMA queues (per-queue descriptor rate is the startup wall)
            for k in range(KT):
                nc.sync.dma_start(wv[k][:, 0:256], wv_d[:, k, 0:256])
                nc.sync.dma_start(wv[k][:, 256:512], wv_d[:, k, 256:512])
                nc.sync.dma_start(xb[k][:, 0:256], xb_d[:, k, 0:256])
                nc.sync.dma_start(xb[k][:, 256:512], xb_d[:, k, 256:512])
            for k in range(KT):
                nc.sync.dma_start(wk[k][:], wk_d[:, k, :])
            dma_chunk(1)
            for k in range(KT):
                nc.sync.dma_start(wq[k][:], wq_d[:, k, :])
            dma_chunk(2)
            dma_chunk(3)
            for hp in range(HP):
                nc.sync.dma_start(pw_sb[:, hp, :], pw_d[:, hp, :])

            def v_proj(mt, pool_tag=None):
                pool, tag = pool_tag or (ps2m, "m")
                ps = pool.tile([128, 512], F32, tag=tag, name=f"vp{mt}")
                for k in range(KT):
                    nc.tensor.matmul(
                        ps[:], xb[k][:, mt * 128:(mt + 1) * 128], wv[k][:],
                        start=(k == 0), stop=(k == KT - 1))
                nc.vector.tensor_copy(
                    v_sb[:, mt, :, 0:D],
                    ps[:].rearrange("p (h d) -> p h d", h=HL))

            def qk_proj(dst, w, t, nch, pool_tag=None):
                pool, tag = pool_tag or (ps2m, "m")
                ps = pool.tile([128, 512], F32, tag=tag, name=f"qk{t}_{nch}")
                for k in range(KT):
                    nc.tensor.matmul(
                        ps[:], w[k][:, t * 128:(t + 1) * 128],
                        xb[k][:, nch * 512:(nch + 1) * 512],
                        start=(k == 0), stop=(k == KT - 1))
                nc.vector.tensor_copy(
                    dst[:, t, nch * 512:(nch + 1) * 512], ps[:])

            def outproj(nch, nt, cch, pool_tag=None):
                pool, tag = pool_tag or (ps2m, "m")
                yp = pool.tile([128, 512], F32, tag=tag, name=f"y{nch}_{nt}_{cch}")
                ao = ao_tiles[nch]
                for hp in range(HP):
                    nc.tensor.matmul(
                        yp[:], ao[:, hp, nt * 128:(nt + 1) * 128],
                        pw_sb[:, hp, cch * 512:(cch + 1) * 512],
                        start=(hp == 0), stop=(hp == HP - 1))
                y_sb = py.tile([128, 512], F32, tag="y", name=f"ysb{nch}_{nt}_{cch}")
                # ACT-engine copy ('copy' lives in the exp table set, no
                # swap): keeps the yp-slot recycle chain off the DVE queue,
                # where deferred normalize pieces would delay it ~5us
                nc.scalar.copy(y_sb[:], yp[:])
                nc.sync.dma_start(
                    y_d[nch * 512 + nt * 128:nch * 512 + (nt + 1) * 128,
                        cch * 512:(cch + 1) * 512], y_sb[:])

            # warm-up work emitted densely in DMA-arrival (chunk-major)
            # order: V projection interleaved with t0 K per chunk, then t0 Q
            # for the first chunk; the rest rides as fillers
            for nch in range(NCH):
                for mt in range(4 * nch, 4 * nch + 4):
                    v_proj(mt, (ps2av, "av"))
                qk_proj(k_sb, wk, 0, nch, (ps2av, "av"))
            qk_proj(q_sb, wq, 0, 0, (ps2av, "av"))

            # per-nc filler inventories (drained evenly across that nc's slots)
            nc_fillers = {i: [] for i in range(NCH)}
            for t in range(1, HP):
                for nch in range(NCH):
                    nc_fillers[0].append((qk_proj, (k_sb, wk, t, nch)))
                nc_fillers[0].append((qk_proj, (q_sb, wq, t, 0)))
            for target_nc in range(1, NCH):
                src_nc = target_nc - 1
                for t in range(1, HP):
                    nc_fillers[src_nc].append((qk_proj, (q_sb, wq, t, target_nc)))
                nc_fillers[src_nc].append((qk_proj, (q_sb, wq, 0, target_nc)))

            ao_tiles = {}
            av_tiles = {}
            # P ring: [:, 0] = head 2hp, [:, 1] = head 2hp+1; 16 m-tile
            # slices per side reused each hp (slice-level deps)
            p_ring = pP.tile([128, 2, 16, 512], BF16, tag="pr", name="p_ring")

            units = [(nch, hp, mt) for nch in range(NCH)
                     for hp in range(HP) for mt in range(NT)]
            LAG = 3
            UPN = HP * NT                 # units per n-chunk

            def emit_qk_exp(nch, hp, mt):
                nsl = slice(nch * 512, (nch + 1) * 512)
                msl = slice(mt * 128, (mt + 1) * 128)
                s2 = ps2s.tile([128, 2, 512], F32, tag="s", name=f"s{nch}_{hp}_{mt}")
                nc.tensor.matmul(
                    s2[:, 0], k_sb[0:64, hp, msl], q_sb[0:64, hp, nsl],
                    start=True, stop=True, tile_position=(0, 0))
                nc.tensor.matmul(
                    s2[:, 1], k_sb[64:128, hp, msl], q_sb[64:128, hp, nsl],
                    start=True, stop=True, tile_position=(64, 0))
                # one exp for both heads: ap=1024, out strided over both sides
                nc.scalar.activation(
                    p_ring[:, :, mt, :], s2[:],
                    mybir.ActivationFunctionType.Exp, scale=SCALE)

            def emit_av(nch, hp, mt):
                if (nch, hp) not in av_tiles:
                    av_tiles[(nch, hp)] = (
                        ps2av.tile([128, 512], F32, tag="av", name=f"avA{nch}_{hp}"),
                        ps2av.tile([128, 512], F32, tag="av", name=f"avB{nch}_{hp}"))
                avA, avB = av_tiles[(nch, hp)]
                nc.tensor.matmul(
                    avA[:], v_sb[:, mt, 2 * hp, :], p_ring[:, 0, mt, :],
                    start=(mt == 0), stop=(mt == NT - 1))
                nc.tensor.matmul(
                    avB[:], v_sb[:, mt, 2 * hp + 1, :], p_ring[:, 1, mt, :],
                    start=(mt == 0), stop=(mt == NT - 1))

            def emit_normalize(nch, hp, final=False):
                ao_sb = ao_tiles[nch]
                avA, avB = av_tiles.pop((nch, hp))
                if final:
                    # no one reuses the av PSUM slots after the final pairs:
                    # skip the stash copies, read PSUM directly
                    srcA, srcB = avA, avB
                else:
                    # stash both sides to SBUF immediately: the two copies
                    # free the av PSUM slots the next pair rotates onto; the
                    # slow DVE reciprocals run later in small pieces
                    srcA = pm.tile([128, 512], F32, tag="avsA")
                    srcB = pm.tile([128, 512], F32, tag="avsB")
                    nc.vector.tensor_copy(srcA[:], avA[:])
                    nc.vector.tensor_copy(srcB[:], avB[:])
                rbA = pm.tile([64, 512], F32, tag="rbA")
                rbB = pm.tile([64, 512], F32, tag="rbB")
                # per n-128-slice pieces, mult right after its recips: a
                # consumer of ao[:, hp, q-slice] only waits ~4 pieces
                for q in range(4):
                    qs = slice(q * 128, (q + 1) * 128)
                    for avs, rb, base in ((srcA, rbA, 0), (srcB, rbB, 64)):
                        dve_fillers.append((nch,
                            lambda avs=avs, rb=rb, qs=qs:
                            nc.vector.reciprocal(rb[:, qs], avs[D:2 * D, qs])))
                        dve_fillers.append((nch,
                            lambda avs=avs, rb=rb, base=base, hp=hp, qs=qs:
                            nc.vector.tensor_tensor(
                                ao_sb[base:base + 64, hp, qs], avs[0:D, qs],
                                rb[:, qs], mybir.AluOpType.mult)))
                if hp == HP - 1:
                    if nch < NCH - 1:
                        for nt in range(4):
                            for cch in range(2):
                                nc_fillers[nch + 1].append((outproj, (nch, nt, cch)))
                    else:
                        for nt in range(4):
                            for cch in range(2):
                                nc_fillers[NCH - 1].append((outproj, (nch, nt, cch)))

            drained_nc = {i: 0 for i in range(NCH)}
            dve_fillers = []
            pieces_clear_ui = {}
            for ui, (nch, hp, mt) in enumerate(units):
                if hp == 0 and mt == 0:
                    ao_tiles[nch] = pao.tile([128, HP, 512], F32R, tag="ao",
                                             name=f"ao{nch}")
                # ready work first (AV of lagged unit + fillers), scores last
                if ui >= LAG:
                    lnch, lhp, lmt = units[ui - LAG]
                    emit_av(lnch, lhp, lmt)
                    if lmt == NT - 1:
                        emit_normalize(lnch, lhp)
                # paced filler drain (inventory of the CURRENT nc)
                fl = nc_fillers[nch]
                slot_in_nc = ui % UPN + 1
                inv = len(fl) + drained_nc[nch]
                want = (inv * slot_in_nc + UPN - 1) // UPN
                while fl and drained_nc[nch] < want:
                    f, args = fl.pop(0)
                    if f is outproj and (
                            any(c == args[0] for c, _ in dve_fillers)
                            or ui < pieces_clear_ui.get(args[0], 0) + 2):
                        # outproj reads ao written by deferred normalize
                        # pieces of ITS chunk; hold until those are emitted
                        # AND the DVE has had ~2 units to execute them
                        fl.insert(0, (f, args))
                        break
                    f(*args)
                    drained_nc[nch] += 1
                for _ in range(2):
                    if dve_fillers:
                        c, fn = dve_fillers.pop(0)
                        fn()
                        if not any(cc == c for cc, _ in dve_fillers):
                            pieces_clear_ui[c] = ui
                emit_qk_exp(nch, hp, mt)
            for ui in range(len(units) - LAG, len(units)):
                nch, hp, mt = units[ui]
                emit_av(nch, hp, mt)
                if mt == NT - 1:
                    emit_normalize(nch, hp, final=True)
            while dve_fillers:
                dve_fillers.pop(0)[1]()
            # tail outprojs anchor on the "s"/"av" PSUM tags whose previous
            # users are the final exps/AVs: the WAR on the tile allocation
            # stops walrus hoisting these matmuls early into the PE stream,
            # where their not-yet-written ao stationary would stall the PE
            tail_pools = [(ps2s, "s"), (ps2av, "av")]
            ti = 0
            for i in range(NCH):
                for f, args in nc_fillers[i]:
                    if f is outproj:
                        f(*args, pool_tag=tail_pools[ti % 2])
                        ti += 1
                    else:
                        f(*args)
    return nc


# ---------------------------------------------------------------- entry point

def kernel(x, q_w, k_w, v_mu, v_rho, v_eps, proj_mu, proj_rho, proj_eps,
           pb_mu, pb_rho, pb_eps):
    global LAST_EXEC_TIME_NS
    _ntff_shim()
    import ml_dtypes

    x = np.asarray(x, np.float32)
    v_w = (np.asarray(v_mu, np.float32)
           + _softplus(np.asarray(v_rho)) * np.asarray(v_eps, np.float32))
    p_w = (np.asarray(proj_mu, np.float32)
           + _softplus(np.asarray(proj_rho)) * np.asarray(proj_eps, np.float32))
    p_b = (np.asarray(pb_mu, np.float32)
           + _softplus(np.asarray(pb_rho)) * np.asarray(pb_eps, np.float32))

    def wslice(w, g):  # [128, KT, CL] bf16 layout of w[g*CL:(g+1)*CL, :].T
        wt = np.ascontiguousarray(np.asarray(w, np.float32)[g * CL:(g + 1) * CL, :].T)
        return wt.reshape(KT, 128, CL).transpose(1, 0, 2).astype(ml_dtypes.bfloat16)

    def pwslice(g):    # [128, HP, C] f32r layout of p_w[:, g*CL:(g+1)*CL].T
        pt = np.ascontiguousarray(p_w[:, g * CL:(g + 1) * CL].T)
        return _rne_f32r(pt.reshape(HP, 128, C).transpose(1, 0, 2))

    xts = []
    for b in range(B):
        xt = np.ascontiguousarray(x[b].T)          # [C, N]
        xts.append(xt.reshape(KT, 128, N).transpose(1, 0, 2).astype(ml_dtypes.bfloat16))
    wq = [wslice(q_w, g) for g in range(G)]
    wk = [wslice(k_w, g) for g in range(G)]
    wv = [wslice(v_w, g) for g in range(G)]
    pw = [pwslice(g) for g in range(G)]

    in_maps = []
    for core in range(8):
        b, g = core // 2, core % 2
        in_maps.append({"xb": xts[b], "wq": wq[g], "wk": wk[g],
                        "wv": wv[g], "pw": pw[g]})

    nc = build_nc()
    _split_excess_waits(nc)
    res = bass_utils.run_bass_kernel_spmd(
        nc, in_maps, core_ids=list(range(8)),
        trace=bool(os.environ.get("BASS_TRACE")))
    LAST_EXEC_TIME_NS = res.exec_time_ns

    out = np.empty((B, N, C), np.float32)
    for b in range(B):
        out[b] = res.results[2 * b]["y"] + res.results[2 * b + 1]["y"] + p_b
    return out

